# revision 8
# baseline (speedup 1.0000x reference)
"""Trainium2 Bass kernel for MeshGenLoss (Chamfer + KL + density-uniformity).

Math:
  d[i,j] = |a_i|^2 + |b_j|^2 - 2 a_i.b_j  is computed as ONE K=33 bf16 matmul
  per [128,512] tile: every fp32 scalar is split into 3 exact bf16 limbs, so
  all 9 limb-products of a.b (plus 3 |a|^2 rows against ones, 3 |b|^2 rows)
  accumulate in fp32 PSUM -> fp32-exact distances at bf16 matmul speed.

  Row-min over 4096 cols: ScalarE copies alternate PSUM tiles to SBUF, then
  VectorE tensor_tensor_reduce(min/min) folds TWO tiles per instruction with
  a running-min chained through the reduce initial-value operand.

Sharding: core c owns rows [512c, 512c+512) of each distance matrix
  (pred->target, target->pred, pred->pred self) for both batches = 24 jobs
  of [128 rows x 4096 cols]. For the self matrix the columns are pre-rotated
  by 512c on the host so the masked diagonal always falls in column-tile 0
  (keeps the SPMD program identical across cores); 1e6*I is added there.
"""

import sys

import ml_dtypes
import numpy as np

sys.path.insert(0, "/opt/trn_rl_repo")

B = 2
N = 4096
L = 512
CORES = 8
ROWS = N // CORES  # 512 rows per core
RB = ROWS // 128  # 4 row blocks per core
CT = N // 512  # 8 column tiles per job
K = 33
BF16 = ml_dtypes.bfloat16
BIG = 3.0e38


def _limbs3(x):
    """Split float64 array into 3 bf16 limbs capturing ~24 significand bits."""
    h = x.astype(BF16)
    r = x - h.astype(np.float64)
    m = r.astype(BF16)
    r2 = r - m.astype(np.float64)
    lo = r2.astype(BF16)
    return h, m, lo


def _build_lhsT(a):
    """a: [n, 3] float64 row points -> lhsT [33, n] bf16.

    Rows 0..26: k=(t,p,q) -> -2 * limb_p(a[:, t])  (repeated over q)
    Rows 27..29: limbs of |a|^2
    Rows 30..32: ones (partner of the |b|^2 rhs rows)
    """
    n = a.shape[0]
    asq = (a * a).sum(-1)
    al = _limbs3(a)  # tuple of [n,3] bf16
    sl = _limbs3(asq)
    out = np.zeros((K, n), dtype=BF16)
    k = 0
    for t in range(3):
        for p in range(3):
            row = (-2.0 * al[p][:, t].astype(np.float64)).astype(BF16)
            for _q in range(3):
                out[k] = row
                k += 1
    for p in range(3):
        out[k] = sl[p]
        k += 1
    for _q in range(3):
        out[k] = np.ones(n, dtype=BF16)
        k += 1
    return out


def _build_rhs(b):
    """b: [m, 3] float64 column points -> rhs [33, m] bf16.

    Rows 0..26: k=(t,p,q) -> limb_q(b[:, t])  (repeated over p)
    Rows 27..29: ones (partner of the |a|^2 lhsT rows)
    Rows 30..32: limbs of |b|^2
    """
    m = b.shape[0]
    bsq = (b * b).sum(-1)
    bl = _limbs3(b)
    sl = _limbs3(bsq)
    out = np.zeros((K, m), dtype=BF16)
    k = 0
    for t in range(3):
        for _p in range(3):
            for q in range(3):
                out[k] = bl[q][:, t]
                k += 1
    for _p in range(3):
        out[k] = np.ones(m, dtype=BF16)
        k += 1
    for q in range(3):
        out[k] = sl[q]
        k += 1
    return out


def _build_program():
    import concourse.bacc as bacc
    import concourse.mybir as mybir
    import concourse.tile as tile
    from contextlib import ExitStack

    dt = mybir.dt
    Alu = mybir.AluOpType
    Act = mybir.ActivationFunctionType

    nc = bacc.Bacc("TRN2", target_bir_lowering=False, debug=False)

    d_lhsT_pt = nc.declare_dram_parameter("lhsT_pt", [B, K, ROWS], dt.bfloat16, isOutput=False)
    d_lhsT_tp = nc.declare_dram_parameter("lhsT_tp", [B, K, ROWS], dt.bfloat16, isOutput=False)
    d_rhs_t = nc.declare_dram_parameter("rhs_t", [B, K, N], dt.bfloat16, isOutput=False)
    d_rhs_p = nc.declare_dram_parameter("rhs_p", [B, K, N], dt.bfloat16, isOutput=False)
    d_diag = nc.declare_dram_parameter("diag", [128, 128], dt.float32, isOutput=False)
    d_mu = nc.declare_dram_parameter("mu_sl", [1, 128], dt.float32, isOutput=False)
    d_lv = nc.declare_dram_parameter("lv_sl", [1, 128], dt.float32, isOutput=False)

    o_pt = nc.declare_dram_parameter("o_pt", [B, RB, 128], dt.float32, isOutput=True)
    o_tp = nc.declare_dram_parameter("o_tp", [B, RB, 128], dt.float32, isOutput=True)
    o_pp = nc.declare_dram_parameter("o_pp", [B, RB, 128], dt.float32, isOutput=True)
    o_kl = nc.declare_dram_parameter("o_kl", [1, 3], dt.float32, isOutput=True)
    o_map = {"pt": o_pt, "tp": o_tp, "pp": o_pp}

    with tile.TileContext(nc) as tc, ExitStack() as ctx:
        consts = ctx.enter_context(tc.tile_pool(name="consts", bufs=1))
        psum = ctx.enter_context(tc.tile_pool(name="psum", bufs=2, space="PSUM"))
        cpool = ctx.enter_context(tc.tile_pool(name="cp", bufs=6))
        apool = ctx.enter_context(tc.tile_pool(name="acc", bufs=16))

        # ---- resident inputs -----------------------------------------
        lhsT_sb = {}
        rhs_sb = {}
        for b in range(B):
            t1 = consts.tile([K, ROWS], dt.bfloat16, tag=f"lpt{b}")
            nc.sync.dma_start(out=t1[:], in_=d_lhsT_pt[b])
            lhsT_sb["pt", b] = t1
            lhsT_sb["pp", b] = t1
            t2 = consts.tile([K, ROWS], dt.bfloat16, tag=f"ltp{b}")
            nc.sync.dma_start(out=t2[:], in_=d_lhsT_tp[b])
            lhsT_sb["tp", b] = t2
            r1 = consts.tile([K, N], dt.bfloat16, tag=f"rt{b}")
            nc.sync.dma_start(out=r1[:], in_=d_rhs_t[b])
            rhs_sb["pt", b] = r1
            r2 = consts.tile([K, N], dt.bfloat16, tag=f"rp{b}")
            nc.sync.dma_start(out=r2[:], in_=d_rhs_p[b])
            rhs_sb["tp", b] = r2
            rhs_sb["pp", b] = r2
        diag_sb = consts.tile([128, 128], dt.float32, tag="diag")
        nc.sync.dma_start(out=diag_sb[:], in_=d_diag[:])
        mu_sb = consts.tile([1, 128], dt.float32, tag="mu")
        nc.sync.dma_start(out=mu_sb[:], in_=d_mu[:])
        lv_sb = consts.tile([1, 128], dt.float32, tag="lv")
        nc.sync.dma_start(out=lv_sb[:], in_=d_lv[:])

        # ---- KL partials ---------------------------------------------
        s1 = apool.tile([1, 1], dt.float32, tag="kls")
        nc.vector.tensor_reduce(s1[:], lv_sb[:], axis=mybir.AxisListType.X, op=Alu.add)
        e_t = consts.tile([1, 128], dt.float32, tag="klexp")
        s3 = apool.tile([1, 1], dt.float32, tag="kls")
        nc.scalar.activation(e_t[:], lv_sb[:], Act.Exp, accum_out=s3[:])
        sq_t = consts.tile([1, 128], dt.float32, tag="klsq")
        s2 = apool.tile([1, 1], dt.float32, tag="kls")
        nc.scalar.activation(sq_t[:], mu_sb[:], Act.Square, accum_out=s2[:])
        nc.sync.dma_start(out=o_kl[0, 0:1], in_=s1[:, 0])
        nc.sync.dma_start(out=o_kl[0, 1:2], in_=s2[:, 0])
        nc.sync.dma_start(out=o_kl[0, 2:3], in_=s3[:, 0])

        # ---- 24 distance-matrix jobs ---------------------------------
        # Two job flavors balance DVE vs ACT:
        #  A: ScalarE copies both PSUM chunks to bf16 SBUF; VectorE does a
        #     pure-bf16 min tree (2x DVE mode).
        #  B: ScalarE copies only chunk1; VectorE's level-0 min reads chunk0
        #     straight from PSUM (1x).
        jobs = [(b, r, kind) for b in range(B) for r in range(RB)
                for kind in ("pt", "tp", "pp")]
        for jidx, (b, r, kind) in enumerate(jobs):
            lhsT = lhsT_sb[kind, b][:, 128 * r:128 * (r + 1)]
            rhs = rhs_sb[kind, b]
            chunks = []
            for h in range(2):
                ch = psum.tile([128, 2048], dt.float32, tag="ps")
                for t in range(4):
                    nc.tensor.matmul(
                        ch[:, 512 * t:512 * (t + 1)],
                        lhsT, rhs[:, 2048 * h + 512 * t:2048 * h + 512 * (t + 1)],
                        start=True, stop=True,
                    )
                chunks.append(ch)
            if kind == "pp":
                # mask the self-distance diagonal (always in chunk 0 at
                # offset 128*r thanks to the host-side column rotation)
                sl = chunks[0][:, 128 * r:128 * r + 128]
                nc.vector.tensor_tensor(sl, sl, diag_sb[:], Alu.add)
            a_type = jidx % 4 != 3  # 18 of 24 jobs
            cb1 = cpool.tile([128, 2048], dt.bfloat16, tag="cp")
            nc.scalar.copy(cb1[:], chunks[1][:])
            if a_type:
                cb0 = cpool.tile([128, 2048], dt.bfloat16, tag="cp")
                nc.scalar.copy(cb0[:], chunks[0][:])
                nc.vector.tensor_tensor(cb1[:], cb0[:], cb1[:], Alu.min)
            else:
                nc.vector.tensor_tensor(cb1[:], chunks[0][:], cb1[:], Alu.min)
            nc.vector.tensor_tensor(
                cb1[:, :1024], cb1[:, :1024], cb1[:, 1024:], Alu.min)
            nc.vector.tensor_tensor(
                cb1[:, :512], cb1[:, :512], cb1[:, 512:1024], Alu.min)
            acc = apool.tile([128, 1], dt.float32, tag="acc")
            nc.vector.tensor_reduce(
                acc[:], cb1[:, :512], axis=mybir.AxisListType.X, op=Alu.min)
            nc.sync.dma_start(out=o_map[kind][b, r, :], in_=acc[:, 0])

    nc.compile()
    return nc


def _make_in_maps(pred, target, mu, logvar):
    pred = np.asarray(pred, dtype=np.float32)
    target = np.asarray(target, dtype=np.float32)
    mu = np.asarray(mu, dtype=np.float32)
    logvar = np.asarray(logvar, dtype=np.float32)

    pred64 = pred.astype(np.float64)
    target64 = target.astype(np.float64)

    # Shared (core-independent) operands
    rhs_t = np.stack([_build_rhs(target64[b]) for b in range(B)])  # [B,K,N]
    rhs_p_full = np.stack([_build_rhs(pred64[b]) for b in range(B)])
    diag = (np.eye(128, dtype=np.float32) * 1.0e6)
    mu_flat = mu.reshape(-1)
    lv_flat = logvar.reshape(-1)

    in_maps = []
    for c in range(CORES):
        rows = slice(ROWS * c, ROWS * (c + 1))
        lhsT_pt = np.stack([_build_lhsT(pred64[b, rows]) for b in range(B)])
        lhsT_tp = np.stack([_build_lhsT(target64[b, rows]) for b in range(B)])
        rot = np.roll(rhs_p_full, -ROWS * c, axis=2)
        in_maps.append({
            "lhsT_pt": lhsT_pt,
            "lhsT_tp": lhsT_tp,
            "rhs_t": rhs_t,
            "rhs_p": np.ascontiguousarray(rot),
            "diag": diag,
            "mu_sl": mu_flat[128 * c:128 * (c + 1)].reshape(1, 128),
            "lv_sl": lv_flat[128 * c:128 * (c + 1)].reshape(1, 128),
        })
    return in_maps


def kernel(pred, target, mu, logvar):
    from concourse.bass_utils import run_bass_kernel_spmd

    in_maps = _make_in_maps(pred, target, mu, logvar)
    nc = _build_program()
    res = run_bass_kernel_spmd(nc, in_maps, list(range(CORES)))
    results = res.results

    nn_pt = np.concatenate([r["o_pt"].reshape(B, ROWS) for r in results], axis=1)
    nn_tp = np.concatenate([r["o_tp"].reshape(B, ROWS) for r in results], axis=1)
    nn_pp = np.concatenate([r["o_pp"].reshape(B, ROWS) for r in results], axis=1)
    kl_parts = np.stack([r["o_kl"].reshape(3) for r in results])  # [CORES,3]

    nn_pt64 = nn_pt.astype(np.float64)
    nn_tp64 = nn_tp.astype(np.float64)
    nn_pp64 = nn_pp.astype(np.float64)

    cd = (nn_pt64.mean(axis=1) + nn_tp64.mean(axis=1)).mean()

    s1 = kl_parts[:, 0].astype(np.float64).sum()
    s2 = kl_parts[:, 1].astype(np.float64).sum()
    s3 = kl_parts[:, 2].astype(np.float64).sum()
    n_kl = B * L
    kl = -0.5 * (n_kl + s1 - s2 - s3) / n_kl

    density = np.std(nn_pp64, axis=1, ddof=1).mean()

    total = cd + 0.001 * kl + 0.1 * density

    return (
        np.float32(total),
        np.float32(cd),
        np.float32(kl),
        np.float32(density),
    )
